# revision 2
# baseline (speedup 1.0000x reference)
"""Trainium2 Bass kernel for nn_AttentionBlock (B=4, H=W=64, C=256), SPMD over 8 NeuronCores.

Strategy:
  - Shard: batch b = core//2, query-half = core%2 (2048 queries/core, full 4096 keys).
    Key order is permuted per-core (own-half first) — softmax is permutation-invariant.
  - All matmuls in float32r (tf32): host pre-rounds inputs; PE multiplies exactly with
    fp32 PSUM accumulation (verified: only error source is the tf32 input rounding).
  - Transposed dataflow: K^T/Q^T [c, pos]; scores computed as S^T [kk, q] so softmax's
    key-reduction becomes a PE ones-matmul; exp via ACT with per-partition bias
    tau[kk] = <bq/16, K[kk]> (the only part of the q/k biases softmax doesn't cancel).
  - V natural layout [kk, c] feeds attn@V directly: O'^T[c,q] accumulated over 32 kk tiles.
  - Device returns Y^T = wp.T @ O'^T (unnormalized) + denominators; host divides,
    adds bv@wp + bp and the residual x.
"""
import numpy as np

B, HH, WW, C = 4, 64, 64, 256
HW = HH * WW          # 4096 spatial positions
QH = HW // 2          # 2048 queries per core
NC = 8
KT = HW // 128        # 32 kk tiles
QCH = QH // 512       # 4 query chunks of 512
# blob columns: xT | wq/16 | wk | wv | wp | bq/16
XO, WQO, WKO, WVO, WPO, BQO = 0, HW, HW + C, HW + 2 * C, HW + 3 * C, HW + 4 * C
WBLOB = HW + 4 * C + 2  # 5122 (bq stored twice: fp32r matmul needs even N)


def tf32_round(x: np.ndarray) -> np.ndarray:
    u = np.ascontiguousarray(x, np.float32).view(np.uint32).astype(np.uint64)
    u = (u + 0x1000 + ((u >> 13) & 1)) & 0xFFFFE000
    return u.astype(np.uint32).view(np.float32)


def build_nc():
    import concourse.bass as bass  # noqa: F401
    import concourse.tile as tile
    from concourse import bacc, mybir

    f32 = mybir.dt.float32
    f32r = mybir.dt.float32r
    AF = mybir.ActivationFunctionType

    nc = bacc.Bacc("TRN2", target_bir_lowering=False, debug=False, num_devices=NC)
    blob = nc.dram_tensor("blob", [2, 128, WBLOB], f32r, kind="ExternalInput").ap()
    y_out = nc.dram_tensor("y", [2, 128, QH], f32, kind="ExternalOutput").ap()
    den_out = nc.dram_tensor("den", [1, QH], f32, kind="ExternalOutput").ap()

    with tile.TileContext(nc) as tc:
        with tc.tile_pool(name="sb", bufs=1) as sb, \
             tc.tile_pool(name="pp", bufs=8) as pp, \
             tc.tile_pool(name="psA", bufs=4, space="PSUM") as psA, \
             tc.tile_pool(name="psO", bufs=1, space="PSUM") as psO, \
             tc.tile_pool(name="psD", bufs=1, space="PSUM") as psD:
            bl = sb.tile([128, 2, WBLOB], f32r)
            nc.sync.dma_start(bl[:, 0], blob[0])
            nc.sync.dma_start(bl[:, 1], blob[1])
            xT = bl[:, :, XO:XO + HW]
            wq = bl[:, :, WQO:WQO + C]
            wk = bl[:, :, WKO:WKO + C]
            wv = bl[:, :, WVO:WVO + C]
            wp = bl[:, :, WPO:WPO + C]
            bq = bl[:, :, BQO:BQO + 2]

            # --- K^T [c(2x128), kk 4096] ---
            kTt = sb.tile([128, 2, HW], f32r)
            for m in range(2):
                for n in range(8):
                    ps = psA.tile([128, 512], f32, tag="ps", name=f"psk{m}{n}")
                    for k in range(2):
                        nc.tensor.matmul(ps, wk[:, k, m * 128:(m + 1) * 128],
                                         xT[:, k, n * 512:(n + 1) * 512],
                                         start=(k == 0), stop=(k == 1))
                    nc.scalar.activation(kTt[:, m, n * 512:(n + 1) * 512], ps, AF.Identity)

            # --- tau[kk] = K^T.T @ (bq/16): per-kk softmax bias term ---
            pst = psA.tile([128, 2 * KT], f32, tag="tau", bufs=1, name="pstau")
            for t in range(KT):
                for m in range(2):
                    nc.tensor.matmul(pst[:, 2 * t:2 * t + 2], kTt[:, m, t * 128:(t + 1) * 128],
                                     bq[:, m], start=(m == 0), stop=(m == 1),
                                     skip_group_check=True)
            tau = sb.tile([128, 2 * KT], f32)
            nc.scalar.activation(tau, pst, AF.Identity)

            # --- Q^T [c(2x128), q 2048] (queries are xT cols 0:QH) ---
            qTt = sb.tile([128, 2, QH], f32r)
            for m in range(2):
                for n in range(QCH):
                    ps = psA.tile([128, 512], f32, tag="ps", name=f"psq{m}{n}")
                    for k in range(2):
                        nc.tensor.matmul(ps, wq[:, k, m * 128:(m + 1) * 128],
                                         xT[:, k, n * 512:(n + 1) * 512],
                                         start=(k == 0), stop=(k == 1))
                    nc.scalar.activation(qTt[:, m, n * 512:(n + 1) * 512], ps, AF.Identity)

            # --- V [kk(32x128), c 256] ---
            vt = sb.tile([128, KT, C], f32r)
            for t in range(KT):
                ps = psA.tile([128, C], f32, tag="ps", name=f"psv{t}")
                for k in range(2):
                    nc.tensor.matmul(ps, xT[:, k, t * 128:(t + 1) * 128], wv[:, k],
                                     start=(k == 0), stop=(k == 1))
                nc.scalar.activation(vt[:, t], ps, AF.Identity)

            ones_t = sb.tile([128, 1], f32r)
            nc.scalar.activation(ones_t, tau[:, 0:1], AF.Identity, scale=0.0, bias=1.0)

            # --- attention main loop over query chunks ---
            oT = sb.tile([128, 2, QH], f32r)
            den_s = sb.tile([1, QH], f32)
            for j in range(QCH):
                po0 = psO.tile([128, 512], f32, tag="o0", name=f"po0_{j}")
                po1 = psO.tile([128, 512], f32, tag="o1", name=f"po1_{j}")
                pd = psD.tile([1, 512], f32, tag="d", name=f"pd_{j}")
                for t in range(KT):
                    ps = psA.tile([128, 512], f32, tag="ps", name=f"pss{j}_{t}")
                    for m in range(2):
                        nc.tensor.matmul(ps, kTt[:, m, t * 128:(t + 1) * 128],
                                         qTt[:, m, j * 512:(j + 1) * 512],
                                         start=(m == 0), stop=(m == 1))
                    pT = pp.tile([128, 512], f32r, tag="p", name=f"pt{j}_{t}")
                    nc.scalar.activation(pT, ps, AF.Exp, bias=tau[:, 2 * t:2 * t + 1], scale=1.0)
                    nc.tensor.matmul(po0, vt[:, t, 0:128], pT,
                                     start=(t == 0), stop=(t == KT - 1),
                                     skip_group_check=True)
                    nc.tensor.matmul(po1, vt[:, t, 128:256], pT,
                                     start=(t == 0), stop=(t == KT - 1),
                                     skip_group_check=True)
                    nc.tensor.matmul(pd, ones_t, pT,
                                     start=(t == 0), stop=(t == KT - 1),
                                     skip_group_check=True)
                nc.scalar.activation(oT[:, 0, j * 512:(j + 1) * 512], po0, AF.Identity)
                nc.scalar.activation(oT[:, 1, j * 512:(j + 1) * 512], po1, AF.Identity)
                nc.scalar.activation(den_s[0:1, j * 512:(j + 1) * 512], pd, AF.Identity)

            # --- final projection Y^T = wp.T @ O'^T ---
            y_s = sb.tile([128, 2, QH], f32)
            for m in range(2):
                for n in range(QCH):
                    ps = psA.tile([128, 512], f32, tag="ps", name=f"psy{m}{n}")
                    for k in range(2):
                        nc.tensor.matmul(ps, wp[:, k, m * 128:(m + 1) * 128],
                                         oT[:, k, n * 512:(n + 1) * 512],
                                         start=(k == 0), stop=(k == 1))
                    nc.scalar.activation(y_s[:, m, n * 512:(n + 1) * 512], ps, AF.Identity)
            nc.sync.dma_start(y_out[0], y_s[:, 0])
            nc.sync.dma_start(y_out[1], y_s[:, 1])
            nc.sync.dma_start(den_out, den_s)
    nc.compile()
    return nc


def make_in_maps(x, wq, bq, wk, wv, wp):
    """Per-core input blobs. x: [B,H,W,C] float32."""
    xf = np.ascontiguousarray(x, np.float32).reshape(B, HW, C)
    wqs = np.ascontiguousarray(wq, np.float32) / 16.0
    bqs = np.ascontiguousarray(bq, np.float32) / 16.0
    in_maps = []
    for c in range(NC):
        b, h = divmod(c, 2)
        X = xf[b]
        xP = np.concatenate([X[h * QH:(h + 1) * QH], X[(1 - h) * QH:(2 - h) * QH]], axis=0)
        blob = np.empty((C, WBLOB), np.float32)
        blob[:, XO:XO + HW] = xP.T
        blob[:, WQO:WQO + C] = wqs
        blob[:, WKO:WKO + C] = np.asarray(wk, np.float32)
        blob[:, WVO:WVO + C] = np.asarray(wv, np.float32)
        blob[:, WPO:WPO + C] = np.asarray(wp, np.float32)
        blob[:, BQO] = bqs
        blob[:, BQO + 1] = bqs
        in_maps.append({"blob": tf32_round(blob.reshape(2, 128, WBLOB))})
    return in_maps


def postprocess(results, x, bq, bk, bv, bp, wp):
    """Assemble full output from per-core Y^T + denominators."""
    xf = np.ascontiguousarray(x, np.float32).reshape(B, HW, C)
    bvp = (np.asarray(bv, np.float64) @ np.asarray(wp, np.float64) +
           np.asarray(bp, np.float64)).astype(np.float32)
    out = np.empty((B, HW, C), np.float32)
    for c in range(NC):
        b, h = divmod(c, 2)
        yT = results[c]["y"].reshape(C, QH)          # [256, 2048]
        den = results[c]["den"].reshape(QH)          # [2048]
        rows = yT.T / den[:, None] + bvp[None, :] + xf[b, h * QH:(h + 1) * QH]
        out[b, h * QH:(h + 1) * QH] = rows
    return out.reshape(B, HH, WW, C)


_NC_CACHE = None


def _get_nc():
    global _NC_CACHE
    if _NC_CACHE is None:
        _NC_CACHE = build_nc()
    return _NC_CACHE


def kernel(x, t, wq, bq, wk, bk, wv, bv, wp, bp):
    from concourse.bass_utils import run_bass_kernel_spmd
    in_maps = make_in_maps(x, wq, bq, wk, wv, wp)
    nc = _get_nc()
    res = run_bass_kernel_spmd(nc, in_maps, core_ids=list(range(NC)))
    return postprocess(res.results, x, bq, bk, bv, bp, wp)
